# revision 1
# baseline (speedup 1.0000x reference)
"""ChemProp msg-to-node kernel for 8 Trainium2 NeuronCores.

reference:
    msg = segment_sum(h[800000, 96], nbrs[:, 0], num_segments=100000)
    out = relu(concat([r[100000, 128], msg], axis=1) @ W_out[96, 224].T)

Strategy (shard by destination node; fully local segment-sum per core):
  - Host: assign nodes to 800 "windows" of <=128 node slots, balanced by
    degree so each window owns <=1024 incoming edges (serpentine deal on
    degree-sorted nodes + greedy repair).  Windows 100*c..100*(c+1)-1 go
    to core c.  Edges are grouped per window and padded to 1024 slots
    (zero message rows contribute nothing to the scatter-sum).
  - Device (per window): one-hot scatter matrices M_j[e, c] =
    (dst_rel[e] == c) built on DVE via iota-compare; 8 accumulating
    matmuls compute msgT[96, 128] = sum_j h_j.T @ M_j in PSUM; two more
    matmuls apply the fused Linear: outT = WrT.T @ rT + WmT.T @ msgT;
    ReLU on ScalarE; staged output DMA.
  - Host: gather per-core [96, 12800] outputs, transpose, inverse-permute.
"""

import numpy as np

N_NODES = 100000
N_EDGES = 800000
D_R = 128
D_H = 96
D_OUT = 96
NCORES = 8

NW = 128          # node slots per window
CAP = 1024        # edge capacity per window
CPW = CAP // 128  # chunks of 128 edges per window = 8
GROUP = 10        # windows per output/rT staging group

_WAIT_LIMIT = 1   # walrus CoreV3 allows a single sync-wait per instruction


def _split_sync_waits(nc, mybir, limit=_WAIT_LIMIT):
    """Move overflow sem-waits onto no-ops just before the offending
    instruction (same engine, same block => runs earlier in program
    order, so all waits still complete before the instruction)."""
    n_new = 0
    for fn in nc.m.functions:
        for bb in fn.blocks:
            out = []
            changed = False
            for inst in bb.instructions:
                si = getattr(inst, "sync_info", None)
                waits = list(si.on_wait) if (si is not None and si.on_wait) else []
                if len(waits) > limit:
                    head, tail = waits[:-limit], waits[-limit:]
                    for k in range(0, len(head), limit):
                        nop = mybir.InstNoOp(
                            name=f"{inst.name}-wsplit{n_new}", ins=[], outs=[]
                        )
                        nop.engine = inst.engine
                        nop.sync_info = mybir.SyncInfo(
                            on_wait=head[k : k + limit], on_update=[]
                        )
                        out.append(nop)
                        n_new += 1
                    si.on_wait = tail
                    changed = True
                out.append(inst)
            if changed:
                bb.instructions.clear()
                bb.instructions.extend(out)
    return n_new


def _pack_nodes(deg, w_total):
    """Assign each node to a (window, slot) so every window has <=NW nodes
    and total degree <=CAP.  Serpentine deal on degree-sorted nodes, plus a
    greedy repair pass for any window that still exceeds CAP."""
    n = deg.shape[0]
    order = np.argsort(-deg, kind="stable")
    win_of_node = np.empty(n, dtype=np.int64)
    slot_of_node = np.empty(n, dtype=np.int64)
    rounds = (n + w_total - 1) // w_total
    for rnd in range(rounds):
        lo = rnd * w_total
        hi = min(lo + w_total, n)
        chunk = order[lo:hi]
        wins = np.arange(hi - lo)
        if rnd % 2 == 1:
            wins = w_total - 1 - wins
        win_of_node[chunk] = wins
        slot_of_node[chunk] = rnd
    loads = np.bincount(win_of_node, weights=deg, minlength=w_total).astype(np.int64)
    counts = np.bincount(win_of_node, minlength=w_total)
    if loads.max() > CAP:
        # greedy repair: move nodes out of overloaded windows
        win_nodes = [list(np.where(win_of_node == w)[0]) for w in range(w_total)]
        for w in np.where(loads > CAP)[0]:
            nodes = sorted(win_nodes[w], key=lambda x: -deg[x])
            for nd in nodes:
                if loads[w] <= CAP:
                    break
                cands = np.where((counts < NW) & (loads + deg[nd] <= CAP))[0]
                cands = cands[cands != w]
                if len(cands) == 0:
                    raise RuntimeError("window packing failed")
                tgt = cands[np.argmin(loads[cands])]
                loads[w] -= deg[nd]
                loads[tgt] += deg[nd]
                counts[w] -= 1
                counts[tgt] += 1
                win_of_node[nd] = tgt
                win_nodes[tgt].append(nd)
        # recompute slots per window
        for w in range(w_total):
            nds = np.where(win_of_node == w)[0]
            slot_of_node[nds] = np.arange(len(nds))
    assert counts.max() <= NW and loads.max() <= CAP
    return win_of_node, slot_of_node


def _build_bass(w_pc, reps=1):
    import concourse.bass as bass
    import concourse.tile as tile
    from concourse import mybir

    f32 = mybir.dt.float32
    nc = bass.Bass()
    h_d = nc.declare_dram_parameter("h", [w_pc, 128, CPW * D_H], f32, isOutput=False)
    dst_d = nc.declare_dram_parameter("dstrel", [128, w_pc * CPW], f32, isOutput=False)
    rT_d = nc.declare_dram_parameter("rT", [128, w_pc * NW], f32, isOutput=False)
    wrT_d = nc.declare_dram_parameter("wrT", [D_R, D_OUT], f32, isOutput=False)
    wmT_d = nc.declare_dram_parameter("wmT", [D_H, D_OUT], f32, isOutput=False)
    out_d = nc.declare_dram_parameter("out", [D_OUT, w_pc * NW], f32, isOutput=True)

    n_groups = w_pc // GROUP
    assert w_pc % GROUP == 0

    with tile.TileContext(nc) as tc:
        with (
            tc.tile_pool(name="const", bufs=1) as const,
            tc.tile_pool(name="hp", bufs=3) as hp,
            tc.tile_pool(name="mp", bufs=2) as mp,
            tc.tile_pool(name="rp", bufs=2) as rp,
            tc.tile_pool(name="op", bufs=2) as op,
            tc.tile_pool(name="sp", bufs=2) as sp,
            tc.tile_pool(name="ps_m", bufs=2, space="PSUM") as ps_m,
            tc.tile_pool(name="ps_o", bufs=2, space="PSUM") as ps_o,
        ):
            iota_i = const.tile([128, 128], mybir.dt.int32)
            nc.gpsimd.iota(iota_i[:], pattern=[[1, 128]], base=0, channel_multiplier=0)
            iota_t = const.tile([128, 128], f32)
            nc.vector.tensor_copy(iota_t[:], iota_i[:])
            wrT_t = const.tile([D_R, D_OUT], f32)
            nc.sync.dma_start(wrT_t[:], wrT_d[:])
            wmT_t = const.tile([D_H, D_OUT], f32)
            nc.sync.dma_start(wmT_t[:], wmT_d[:])
            dst_t = const.tile([128, w_pc * CPW], f32)
            nc.sync.dma_start(dst_t[:], dst_d[:])

            for _rep in range(reps):
                for g in range(n_groups):
                    rt = rp.tile([128, GROUP * NW], f32)
                    nc.sync.dma_start(
                        rt[:], rT_d[:, g * GROUP * NW : (g + 1) * GROUP * NW]
                    )
                    ot = op.tile([D_OUT, GROUP * NW], f32)
                    for wl in range(GROUP):
                        w = g * GROUP + wl
                        ht = hp.tile([128, CPW * D_H], f32)
                        nc.sync.dma_start(ht[:], h_d[w])
                        m = mp.tile([128, CPW * 128], f32)
                        msum = ps_m.tile([D_H, NW], f32)
                        for j in range(CPW):
                            nc.vector.tensor_scalar(
                                m[:, j * 128 : (j + 1) * 128],
                                iota_t[:],
                                dst_t[:, w * CPW + j : w * CPW + j + 1],
                                None,
                                op0=mybir.AluOpType.is_equal,
                            )
                            nc.tensor.matmul(
                                out=msum[:],
                                lhsT=ht[:, j * D_H : (j + 1) * D_H],
                                rhs=m[:, j * 128 : (j + 1) * 128],
                                start=(j == 0),
                                stop=(j == CPW - 1),
                            )
                        msb = sp.tile([D_H, NW], f32)
                        nc.scalar.copy(msb[:], msum[:])
                        osum = ps_o.tile([D_OUT, NW], f32)
                        nc.tensor.matmul(
                            out=osum[:],
                            lhsT=wrT_t[:],
                            rhs=rt[:, wl * NW : (wl + 1) * NW],
                            start=True,
                            stop=False,
                        )
                        nc.tensor.matmul(
                            out=osum[:],
                            lhsT=wmT_t[:],
                            rhs=msb[:],
                            start=False,
                            stop=True,
                        )
                        nc.scalar.activation(
                            ot[:, wl * NW : (wl + 1) * NW],
                            osum[:],
                            mybir.ActivationFunctionType.Relu,
                        )
                    nc.sync.dma_start(
                        out_d[:, g * GROUP * NW : (g + 1) * GROUP * NW], ot[:]
                    )

    _split_sync_waits(nc, mybir)
    return nc


def _prepare(r, h, nbrs, W_out, w_total):
    """Host-side sharding: returns per-core input maps + slot->node map."""
    w_pc = w_total // NCORES
    dst = np.asarray(nbrs)[:, 0].astype(np.int64)
    deg = np.bincount(dst, minlength=N_NODES)
    win_of_node, slot_of_node = _pack_nodes(deg, w_total)

    # edge -> (window, position within window)
    w_e = win_of_node[dst]
    order_e = np.argsort(w_e, kind="stable")
    w_sorted = w_e[order_e]
    counts = np.bincount(w_sorted, minlength=w_total)
    starts = np.zeros(w_total + 1, dtype=np.int64)
    np.cumsum(counts, out=starts[1:])
    pos = np.arange(N_EDGES, dtype=np.int64) - starts[w_sorted]

    e_ids = np.full((w_total, CAP), -1, dtype=np.int64)
    e_ids[w_sorted, pos] = order_e
    valid = e_ids >= 0
    e_clip = np.where(valid, e_ids, 0)

    h = np.ascontiguousarray(np.asarray(h, dtype=np.float32))
    h_slots = h[e_clip.reshape(-1)].reshape(w_total, CAP, D_H)
    h_slots[~valid] = 0.0
    # slot k -> (partition p = k // CPW, chunk j = k % CPW)
    h_dev = h_slots.reshape(w_total, 128, CPW * D_H)

    dst_rel = np.where(valid, slot_of_node[dst[e_clip]], 0).astype(np.float32)
    dst_dev = dst_rel.reshape(w_total, 128, CPW)

    # node -> slot tables for r / output
    node_of_slot = np.full((w_total, NW), -1, dtype=np.int64)
    node_of_slot[win_of_node, slot_of_node] = np.arange(N_NODES)
    r = np.asarray(r, dtype=np.float32)
    r_slots = np.zeros((w_total, NW, D_R), dtype=np.float32)
    nos_valid = node_of_slot >= 0
    r_slots[nos_valid] = r[node_of_slot[nos_valid]]

    W_out = np.asarray(W_out, dtype=np.float32)
    wrT = np.ascontiguousarray(W_out[:, :D_R].T)
    wmT = np.ascontiguousarray(W_out[:, D_R:].T)

    in_maps = []
    for c in range(NCORES):
        ws = slice(c * w_pc, (c + 1) * w_pc)
        dst_c = np.ascontiguousarray(
            dst_dev[ws].transpose(1, 0, 2).reshape(128, w_pc * CPW)
        )
        rT_c = np.ascontiguousarray(
            r_slots[ws].reshape(w_pc * NW, D_R).T
        )
        in_maps.append(
            {
                "h": np.ascontiguousarray(h_dev[ws]),
                "dstrel": dst_c,
                "rT": rT_c,
                "wrT": wrT,
                "wmT": wmT,
            }
        )
    return in_maps, node_of_slot


def kernel(r, h, nbrs, W_out, reps=1, _timing=None):
    from concourse.bass_utils import run_bass_kernel_spmd

    w_total = NCORES * 100
    w_pc = w_total // NCORES
    in_maps, node_of_slot = _prepare(r, h, nbrs, W_out, w_total)
    nc = _build_bass(w_pc, reps=reps)
    res = run_bass_kernel_spmd(nc, in_maps, list(range(NCORES)), trace=False)
    if _timing is not None:
        _timing.append(res)

    outs = [res.results[c]["out"] for c in range(NCORES)]
    outT = np.concatenate(outs, axis=1)          # [96, w_total*NW]
    out_slots = outT.T                            # [w_total*NW, 96]
    node_flat = node_of_slot.reshape(-1)
    result = np.empty((N_NODES, D_OUT), dtype=np.float32)
    m = node_flat >= 0
    result[node_flat[m]] = out_slots[m]
    return result
